# revision 12
# baseline (speedup 1.0000x reference)
"""Trainium2 Bass kernel for nn_DE_NN_67912022884544 (dense_mlp).

Each population l applies a tiny 1->4->8->4->1 ReLU MLP to a scalar input,
pointwise over a 400k-sample batch.  A scalar->scalar ReLU MLP is exactly a
piecewise-linear function of its input, so per population the network
collapses (exactly, in real arithmetic) to

    out(x) = A*x + B + sum_k d_k * relu(x - t_k)

with only ~4-26 knees, computed host-side in float64 from the tiny weights.
Knees outside the observed data range [min X, max X] fold exactly into A, B
(always-active knees are linear over the range; never-active knees vanish).

Device mapping (per core, batch split 8 ways, identical SPMD program):
  * samples ride the 128 SBUF partitions and the free dim; populations are
    packed 4 per tile (32 lanes each); quads are grouped by local search to
    minimize total padded slots sum_q(max_pos + max_neg);
  * per slot: a temp relu(scale*x + bias) [128, 1564] is produced by ScalarE
    (one ACTIVATE with per-partition scale/bias APs) or VectorE (two
    tensor_scalar ops at 2x fp32 mode);
  * accumulation runs on VectorE (tensor_tensor add/sub into an SBUF
    accumulator) with a secondary accumulator fed by GpSimd (Q7
    tensor_tensor), merged once per quad;
  * producer/accumulator engine assignment is balanced from measured
    per-instruction costs so ScalarE / VectorE / GpSimd finish together.
"""

import os

import numpy as np

NP = 44
B = 400000
NCORES = 8
LANES = 32              # sample lanes per population within a 128-partition tile
PPT = 4                 # populations per tile
NQ = NP // PPT          # 11 quads
SHARD = 50048           # per-core samples per population (128*391; 8*SHARD >= B)
FREE = SHARD // LANES   # 1564
RFOLD = 12.0            # fallback |x| bound when data-range pruning is off

# measured per-instruction costs (ns) for engine load balancing, FD=1564 fp32
_C_ACT_RELU = 1680.0    # ScalarE ACTIVATE
_C_DVE_TS = 1040.0      # VectorE tensor_scalar (2x mode)
_C_DVE_TT = 1700.0      # VectorE tensor_tensor (1x)
_C_GPS_TT = 3500.0      # GpSimd tensor_tensor
_C_GPS_CP = 1400.0      # GpSimd tensor_copy

LAST_EXEC_NS = None
LAST_RESULTS = None

_PROGRAM_CACHE = {}


# ---------------------------------------------------------------------------
# Host-side exact PWL decomposition (float64, tiny weights only)
# ---------------------------------------------------------------------------

class _PWL:
    """f(x) = a0*x + b0 + sum d*relu(x - t) over knees [(t, d)]."""

    __slots__ = ("a0", "b0", "knees")

    def __init__(self, a0, b0, knees):
        self.a0 = float(a0)
        self.b0 = float(b0)
        self.knees = sorted(knees)

    def segments(self):
        ts = [t for t, _ in self.knees]
        a, b = self.a0, self.b0
        segs = [(a, b)]
        for t, d in self.knees:
            a += d
            b -= d * t
            segs.append((a, b))
        return [-np.inf] + ts + [np.inf], segs

    def __call__(self, x):
        y = self.a0 * x + self.b0
        for t, d in self.knees:
            y += d * max(x - t, 0.0)
        return y


def _lincomb(fs, ws, bias):
    a0 = sum(w * f.a0 for w, f in zip(ws, fs))
    b0 = sum(w * f.b0 for w, f in zip(ws, fs)) + float(bias)
    kn = {}
    for w, f in zip(ws, fs):
        for t, d in f.knees:
            kn[t] = kn.get(t, 0.0) + w * d
    return _PWL(a0, b0, [(t, d) for t, d in kn.items() if d != 0.0])


def _relu_pwl(f):
    bounds, segs = f.segments()
    kn = {}
    for i, (a, b) in enumerate(segs):
        lo, hi = bounds[i], bounds[i + 1]
        if a != 0.0:
            z = -b / a
            if lo < z < hi:
                kn[z] = kn.get(z, 0.0) + abs(a)
    for t, d in f.knees:
        if f(float(t)) > 0:
            kn[t] = kn.get(t, 0.0) + d
    a0, b0 = segs[0]
    if not (a0 < 0 or (a0 == 0 and b0 > 0)):
        a0, b0 = 0.0, 0.0
    return _PWL(a0, b0, [(t, d) for t, d in kn.items() if d != 0.0])


def _pwl_form(W1, B1, W2, B2, W3, B3, W4, B4, tlo, thi):
    """-> (A, B, [(d, t), ...]) with knees restricted to (tlo, thi)."""
    x_id = _PWL(1.0, 0.0, [])
    h1 = [_relu_pwl(_lincomb([x_id], [W1[i]], B1[i])) for i in range(4)]
    h2 = [_relu_pwl(_lincomb(h1, W2[j], B2[j])) for j in range(8)]
    h3 = [_relu_pwl(_lincomb(h2, W3[k], B3[k])) for k in range(4)]
    out = _lincomb(h3, W4, B4)
    A, Bc = out.a0, out.b0
    terms = []
    for t, d in out.knees:
        if t <= tlo:
            A += d
            Bc += -d * t
        elif t < thi:
            terms.append((d, t))
    return A, Bc, terms


def _group_quads(pos, neg):
    """Partition populations into NQ quads minimizing
    sum_q max(pos) + max(neg): greedy seed + pairwise-swap local search."""
    n = len(pos)
    order = sorted(range(n), key=lambda i: -(pos[i] + neg[i]))
    quads = [order[PPT * q:PPT * q + PPT] for q in range(NQ)]

    def qcost(quad):
        return max(pos[i] for i in quad) + max(neg[i] for i in quad)

    cost = [qcost(qd) for qd in quads]
    improved = True
    while improved:
        improved = False
        for qa in range(NQ):
            for qb in range(qa + 1, NQ):
                for ia in range(PPT):
                    for ib in range(PPT):
                        a, b = quads[qa][ia], quads[qb][ib]
                        quads[qa][ia], quads[qb][ib] = b, a
                        ca, cb = qcost(quads[qa]), qcost(quads[qb])
                        if ca + cb < cost[qa] + cost[qb]:
                            cost[qa], cost[qb] = ca, cb
                            improved = True
                        else:
                            quads[qa][ia], quads[qb][ib] = a, b
    return quads


# ---------------------------------------------------------------------------
# Device program
# ---------------------------------------------------------------------------

def _build_program(sched):
    """sched: per quad, list of slots (producer, accumulator, op) with
    producer in {"act","dve"}, accumulator in {"dve","gps"},
    op in {"add","sub"}."""
    import concourse.bacc as bacc
    import concourse.mybir as mybir
    from concourse.tile import TileContext

    f32 = mybir.dt.float32
    RELU = mybir.ActivationFunctionType.Relu
    MULT, ADD, MAX, SUB = (mybir.AluOpType.mult, mybir.AluOpType.add,
                           mybir.AluOpType.max, mybir.AluOpType.subtract)

    NK = sum(len(s) for s in sched)

    nc = bacc.Bacc("TRN2", target_bir_lowering=False, debug=False,
                   num_devices=NCORES)
    xs = nc.dram_tensor("xs", [NP, SHARD], f32, kind="ExternalInput")
    tab = nc.dram_tensor("tab", [128, 2 * NK + 2 * NQ], f32,
                         kind="ExternalInput")
    ys = nc.dram_tensor("ys", [NP, SHARD], f32, kind="ExternalOutput")

    with TileContext(nc) as tc:
        with tc.tile_pool(name="consts", bufs=1) as cpool, \
             tc.tile_pool(name="xin", bufs=3) as xpool, \
             tc.tile_pool(name="acc", bufs=3) as apool, \
             tc.tile_pool(name="acc2", bufs=3) as a2pool, \
             tc.tile_pool(name="tmp", bufs=10) as tpool:
            tabt = cpool.tile([128, 2 * NK + 2 * NQ], f32)
            nc.sync.dma_start(tabt[:], tab[:, :])
            scratch = cpool.tile([128, 1], f32)
            nc.scalar.activation(scratch[:], tabt[:, 0:1],
                                 mybir.ActivationFunctionType.Copy)
            scratch2 = cpool.tile([128, 1], f32)
            nc.vector.tensor_copy(scratch2[:], tabt[:, 0:1])

            col = 0
            for q in range(NQ):
                xt = xpool.tile([128, FREE], f32)
                src = xs[PPT * q:PPT * (q + 1), :].rearrange(
                    "i (l f) -> (i l) f", l=LANES)
                nc.sync.dma_start(xt[:], src)

                at = apool.tile([128, FREE], f32)
                nc.vector.tensor_scalar(
                    at[:], xt[:],
                    tabt[:, 2 * NK + q:2 * NK + q + 1],
                    tabt[:, 2 * NK + NQ + q:2 * NK + NQ + q + 1],
                    MULT, ADD)

                n_gps = sum(1 for _, a, _ in sched[q] if a == "gps")
                n_cce = sum(1 for _, a, o in sched[q]
                            if a == "cce" and o == "add")
                n_cces = sum(1 for _, a, o in sched[q]
                             if a == "cce" and o == "sub")
                a2 = None
                a3 = None
                a4 = None
                gps_seen = 0
                cce_seen = 0
                cces_seen = 0
                for producer, accum, op in sched[q]:
                    tt = tpool.tile([128, FREE], f32, name=f"t{col}",
                                    tag="tt")
                    sc = tabt[:, col:col + 1]
                    bi = tabt[:, NK + col:NK + col + 1]
                    if producer == "act":
                        nc.scalar.activation(tt[:], xt[:], RELU,
                                             bias=bi, scale=sc)
                    else:
                        nc.vector.tensor_scalar(tt[:], xt[:], sc, bi,
                                                MULT, ADD)
                        nc.vector.tensor_scalar(tt[:], tt[:], 0.0, None, MAX)
                    if accum == "dve":
                        nc.vector.tensor_tensor(
                            at[:], at[:], tt[:], ADD if op == "add" else SUB)
                    elif accum == "cce":
                        # SDMA compute-engine accumulate (add only); sub
                        # terms go to a separate accumulator merged with
                        # a subtract at the end
                        if op == "add":
                            if cce_seen == 0:
                                a3 = a2pool.tile([128, FREE], f32,
                                                 name=f"a3_{q}", tag="a3")
                                nc.gpsimd.dma_start(a3[:], tt[:])
                            else:
                                nc.gpsimd.dma_start(a3[:], tt[:],
                                                    accum_op=ADD)
                            cce_seen += 1
                        else:
                            if cces_seen == 0:
                                a4 = a2pool.tile([128, FREE], f32,
                                                 name=f"a4_{q}", tag="a4")
                                nc.gpsimd.dma_start(a4[:], tt[:])
                            else:
                                nc.gpsimd.dma_start(a4[:], tt[:],
                                                    accum_op=ADD)
                            cces_seen += 1
                    else:
                        if gps_seen == 0:
                            a2 = a2pool.tile([128, FREE], f32,
                                             name=f"a2_{q}", tag="a2")
                            nc.gpsimd.memset(a2[:], 0.0)
                        nc.gpsimd.tensor_tensor(
                            a2[:], a2[:], tt[:],
                            ADD if op == "add" else SUB)
                        gps_seen += 1
                    col += 1
                if n_gps:
                    nc.vector.tensor_tensor(at[:], at[:], a2[:], ADD)
                if n_cce:
                    nc.vector.tensor_tensor(at[:], at[:], a3[:], ADD)
                if n_cces:
                    nc.vector.tensor_tensor(at[:], at[:], a4[:], SUB)

                dst = ys[PPT * q:PPT * (q + 1), :].rearrange(
                    "i (l f) -> (i l) f", l=LANES)
                nc.sync.dma_start(dst, at[:])

    nc.compile()
    return nc


# ---------------------------------------------------------------------------
# Entry point
# ---------------------------------------------------------------------------

def kernel(X, lin1, lin2, lin3, lin4, b1, b2, b3, b4):
    global LAST_EXEC_NS, LAST_RESULTS

    X = np.ascontiguousarray(np.asarray(X, dtype=np.float32))

    if os.environ.get("K_PRUNE", "1") == "1":
        tlo = float(X.min())
        thi = float(X.max())
    else:
        tlo, thi = -RFOLD, RFOLD

    forms = []
    for l in range(NP):
        forms.append(_pwl_form(
            np.asarray(lin1, np.float64)[l, :, 0],
            np.asarray(b1, np.float64)[l, :, 0],
            np.asarray(lin2, np.float64)[l],
            np.asarray(b2, np.float64)[l, :, 0],
            np.asarray(lin3, np.float64)[l],
            np.asarray(b3, np.float64)[l, :, 0],
            np.asarray(lin4, np.float64)[l, 0, :],
            float(np.asarray(b4, np.float64)[l, 0, 0]),
            tlo, thi))

    pos = [sum(1 for d, _ in t if d > 0) for _, _, t in forms]
    neg = [len(t) - p for (_, _, t), p in zip(forms, pos)]
    quads = _group_quads(pos, neg)
    nadd = [max(pos[i] for i in qd) for qd in quads]
    nsub = [max(neg[i] for i in qd) for qd in quads]
    pop_order = [i for qd in quads for i in qd]

    # slot rows: per quad, nadd add-slots then nsub sub-slots; each row is
    # the 4 pops' (scale, bias), zero-padded where a pop has fewer terms
    quad_slot_rows = []
    for q, qd in enumerate(quads):
        ordered = []
        for i in qd:
            _, _, terms = forms[i]
            p = sorted([(d, t) for d, t in terms if d > 0],
                       key=lambda s: s[1])
            m = sorted([(d, t) for d, t in terms if d <= 0],
                       key=lambda s: s[1])
            p += [(0.0, 0.0)] * (nadd[q] - len(p))
            m += [(0.0, 0.0)] * (nsub[q] - len(m))
            ordered.append(p + m)
        rows = []
        for j in range(nadd[q] + nsub[q]):
            op = "add" if j < nadd[q] else "sub"
            row = []
            for slot in range(PPT):
                d, t = ordered[slot][j]
                row.append((abs(d), -abs(d) * t))
            rows.append((row, op))
        quad_slot_rows.append(rows)

    # engine assignment balanced on projected per-engine time
    gps_frac = float(os.environ.get("K_GPSF", "0"))
    n_cce_q = int(os.environ.get("K_CCEQ", "3"))    # CCE add-slots per quad
    n_cces_q = int(os.environ.get("K_CCESQ", "2"))  # CCE sub-slots per quad
    act_ns = 0.0
    dve_ns = NQ * (_C_DVE_TS + _C_DVE_TT * (1 + (n_cces_q > 0)))
    gps_ns = 0.0
    sched = []
    tab_cols = []
    for q in range(NQ):
        slots = quad_slot_rows[q]
        n_add_q = sum(1 for _, op in slots if op == "add")
        n_sub_q = len(slots) - n_add_q
        ngps_q = min(len(slots) - 1, int(round(gps_frac * len(slots))))
        # CCE takes the tails of the add-slot and sub-slot ranges
        cce_lo = max(1, n_add_q - n_cce_q)
        cces_lo = n_add_q + max(1, n_sub_q - n_cces_q)
        qsched = []
        for idx, (row, op) in enumerate(slots):
            if act_ns + _C_ACT_RELU <= dve_ns + 2 * _C_DVE_TS:
                producer = "act"
                act_ns += _C_ACT_RELU
            else:
                producer = "dve"
                dve_ns += 2 * _C_DVE_TS
            if op == "add" and cce_lo <= idx < n_add_q:
                accum = "cce"
            elif op == "sub" and idx >= cces_lo:
                accum = "cce"
            elif idx >= len(slots) - ngps_q:
                accum = "gps"
                gps_ns += _C_GPS_TT
            else:
                accum = "dve"
                dve_ns += _C_DVE_TT
            qsched.append((producer, accum, op))
            tab_cols.append(row)
        sched.append(qsched)

    NK = len(tab_cols)
    tabv = np.zeros((128, 2 * NK + 2 * NQ), dtype=np.float32)
    for col, row in enumerate(tab_cols):
        for slot in range(PPT):
            s_, b_ = row[slot]
            rows_ = slice(slot * LANES, (slot + 1) * LANES)
            tabv[rows_, col] = np.float32(s_)
            tabv[rows_, NK + col] = np.float32(b_)
    for q, qd in enumerate(quads):
        for slot, i in enumerate(qd):
            A, Bc, _ = forms[i]
            rows_ = slice(slot * LANES, (slot + 1) * LANES)
            tabv[rows_, 2 * NK + q] = np.float32(A)
            tabv[rows_, 2 * NK + NQ + q] = np.float32(Bc)

    key = tuple(tuple(s) for s in sched)
    if key not in _PROGRAM_CACHE:
        _PROGRAM_CACHE[key] = _build_program(sched)
    nc = _PROGRAM_CACHE[key]

    Xr = X[pop_order, 0, :]
    Xp = np.zeros((NP, NCORES * SHARD), dtype=np.float32)
    Xp[:, :B] = Xr
    tabv = np.ascontiguousarray(tabv)
    in_maps = [
        {"xs": np.ascontiguousarray(Xp[:, c * SHARD:(c + 1) * SHARD]),
         "tab": tabv}
        for c in range(NCORES)
    ]

    from concourse.bass_utils import run_bass_kernel_spmd
    trace = os.environ.get("K_TRACE", "") == "1"
    res = run_bass_kernel_spmd(nc, in_maps, core_ids=list(range(NCORES)),
                               trace=trace)
    LAST_EXEC_NS = res.exec_time_ns
    LAST_RESULTS = res

    Yr = np.concatenate([res.results[c]["ys"] for c in range(NCORES)],
                        axis=1)[:, :B]
    out = np.empty((NP, 1, B), dtype=np.float32)
    out[pop_order, 0, :] = Yr
    return out
